# revision 55
# baseline (speedup 1.0000x reference)
"""Multi-modality double-value attention on 8 TRN2 NeuronCores.

Sharding: data-parallel over batch (16 items -> 2 per core). Each core runs
the full attention block for its 2 items; weights are replicated. No
collectives. Host pre-transposes x to x^T and casts inputs to bf16; compute
is bf16 with fp32 PSUM accumulation; output is fp32.
"""

import numpy as np
import ml_dtypes

B, N, C = 16, 906, 768
H = 12
D = 64
M1 = 513
N_CORES = 8
BPC = B // N_CORES          # batch items per core
KC = C // 128               # 6 contraction chunks over C
NPAIR = H // 2              # 6 head pairs
NCH = (N + 127) // 128      # 8 key/token chunks over N
KCH = [(i * 128, min(128, N - i * 128)) for i in range(NCH)]
QP = [(0, 512), (512, N - 512)]      # column passes over N
CPASS = [(0, 512), (512, C - 512)]   # column passes over C
SCALE = D ** -0.5
PW = 194  # per-head-pair value block: [V_e(64) | 1 | 1 | 1 | 0*63 | V_o(64)]

TRACE = False          # set by test.py to capture a HW profile
DEBUG_DUMP = False     # add intermediate DRAM outputs (denominators, recips, oT)
LAST_RESULTS = None    # BassKernelResults of the most recent run

_BUILT = None


def _install_trace_shim():
    """The image's antenv lacks axon_hooks; recreate it so trace=True works."""
    import sys, types
    if "antenv.axon_hooks" in sys.modules:
        return
    mod = types.ModuleType("antenv.axon_hooks")
    mod._hook = None
    mod.set_axon_ntff_profile_hook = lambda h: setattr(mod, "_hook", h)
    mod.get_axon_ntff_profile_hook = lambda: mod._hook
    sys.modules["antenv.axon_hooks"] = mod
    import antenv
    antenv.axon_hooks = mod
    from trn_agent_boot.trn_boot import _ntff_profile_via_ctypes
    mod.set_axon_ntff_profile_hook(_ntff_profile_via_ctypes("/opt/axon/libaxon_pjrt.so"))


def _build():
    import concourse.tile as tile
    from concourse import bacc, bass_isa, mybir

    BF = mybir.dt.bfloat16
    F32 = mybir.dt.float32
    AF = mybir.ActivationFunctionType

    nc = bacc.Bacc("TRN2", target_bir_lowering=False, debug=False, num_devices=N_CORES)

    xT_d = nc.dram_tensor("xT", [BPC, C, N], BF, kind="ExternalInput").ap()
    w_d = {
        wn: nc.dram_tensor(wn, [C, C], BF, kind="ExternalInput").ap()
        for wn in ("wq", "wk", "wv", "wvc", "wp")
    }
    bias_d = nc.dram_tensor("bias", [128, C], F32, kind="ExternalInput").ap()
    out_d = nc.dram_tensor("out", [BPC, N, C], F32, kind="ExternalOutput").ap()
    if DEBUG_DUMP:
        dbg_rc = nc.dram_tensor("dbg_rc", [BPC, H, N], F32, kind="ExternalOutput").ap()
        dbg_ot = nc.dram_tensor("dbg_ot", [BPC, NPAIR, 128, N], BF, kind="ExternalOutput").ap()
        dbg_e = nc.dram_tensor("dbg_e", [BPC, 2, NCH, 128, N], BF, kind="ExternalOutput").ap()
        dbg_t1 = nc.dram_tensor("dbg_t1", [BPC, NPAIR, 2, 128, 512], F32, kind="ExternalOutput").ap()
        dbg_v = nc.dram_tensor("dbg_v", [BPC, 3, NCH, 128, NPAIR * PW], BF, kind="ExternalOutput").ap()

    with tile.TileContext(nc) as tc:
        from contextlib import ExitStack
        from concourse import library_config

        with ExitStack() as ctx:
            wpool = ctx.enter_context(tc.tile_pool(name="wpool", bufs=1))
            sb = ctx.enter_context(tc.tile_pool(name="sb", bufs=1))
            ps = ctx.enter_context(tc.tile_pool(name="ps", bufs=1, space="PSUM"))

            # partition_broadcast lives in the gpsimd 'attn' library; the
            # default 'standard' library executes it as garbage on HW
            nc.gpsimd.load_library(library_config.attn)

            # ---- constants: weights + bias ----
            # DMA issue order is tuned so the first projection's operands (wq +
            # item-0 x^T) land first; later weights/items stream in behind them
            w_sb = {}

            def load_w(wn):
                tiles = []
                for kc in range(KC):
                    t = wpool.tile([128, C], BF, name=f"{wn}_{kc}", tag=f"{wn}_{kc}")
                    nc.sync.dma_start(t[:], w_d[wn][kc * 128:(kc + 1) * 128, :])
                    tiles.append(t)
                w_sb[wn] = tiles

            load_w("wq")
            bias_sb = wpool.tile([128, C], F32, name="bias_sb", tag="bias_sb")

            # persistent zero-padded k^T tiles, shared across batch items; the
            # pad halves are zeroed once and never rewritten. Scores run as
            # plain K=128 matmuls (row tiling is ~2x slower per MM on this HW)
            kper = []
            for t_ in range(NPAIR):
                ke = wpool.tile([128, N], BF, name=f"kTh_e{t_}", tag=f"kTh_e{t_}")
                ko = wpool.tile([128, N], BF, name=f"kTh_o{t_}", tag=f"kTh_o{t_}")
                nc.vector.memset(ke[64:128, :], 0.0)
                nc.vector.memset(ko[0:64, :], 0.0)
                kper.append((ke, ko))

            # ---- x^T tiles, both items prefetched ----
            xT = {}
            for kc in range(KC):
                t = sb.tile([128, N], BF, name=f"xT_0_{kc}", tag="xT", bufs=6)
                nc.sync.dma_start(t[:], xT_d[0, kc * 128:(kc + 1) * 128, :])
                xT[(0, kc)] = t
            load_w("wk")
            load_w("wv")
            load_w("wvc")
            for kc in range(KC):
                t = sb.tile([128, N], BF, name=f"xT_1_{kc}", tag="xT", bufs=6)
                nc.sync.dma_start(t[:], xT_d[1, kc * 128:(kc + 1) * 128, :])
                xT[(1, kc)] = t
            load_w("wp")
            nc.sync.dma_start(bias_sb[:], bias_d[:])

            for it in range(BPC):
                pcopy = nc.vector.tensor_copy
                # ============ projections ============
                # v/vc are projected first so that per-pair q/k + attention can
                # start as soon as one pair's q/k are ready -- this pulls the
                # exp stream (the scarcest non-PE resource) much earlier
                v_sb, vc_sb = [], []
                for c, (ts, tsz) in enumerate(KCH):
                    for dst_list, wn, tg in ((v_sb, "wv", "v"), (vc_sb, "wvc", "vc")):
                        dst = sb.tile([128, NPAIR * PW], BF, name=f"{tg}_{it}_{c}",
                                      tag=tg, bufs=NCH + 1)
                        if tsz < 128:
                            # stationary loads may read all 128 partitions; keep
                            # the unwritten tail finite
                            nc.vector.memset(dst[:, :], 0.0)
                        dvw = dst[0:tsz, :].rearrange("p (g c) -> p g c", c=PW)
                        for (cs, cw) in CPASS:
                            pp = ps.tile([128, 512], F32, name="pp", tag="ps_a", bufs=4)
                            for kc in range(KC):
                                nc.tensor.matmul(
                                    pp[0:tsz, 0:cw],
                                    lhsT=xT[(it, kc)][:, ts:ts + tsz],
                                    rhs=w_sb[wn][kc][:, cs:cs + cw],
                                    start=(kc == 0), stop=(kc == KC - 1),
                                )
                            g0, gn = (0, 4) if cs == 0 else (4, 2)
                            src = pp[0:tsz, 0:cw].rearrange("p (g r d) -> p g r d", r=2, d=D)
                            pcopy(dvw[:, g0:g0 + gn, 0:D], src[:, :, 0, :])
                            pcopy(dvw[:, g0:g0 + gn, 130:194], src[:, :, 1, :])
                        # cols 64 and 66 are the denominator ones-columns; cols
                        # 67:130 feed PSUM rows that are never read, so they can
                        # stay stale
                        nc.vector.memset(dvw[:, :, 64:67], 1.0)
                        dst_list.append(dst)

                # mixed tiles for the key chunk straddling M1 (chunk 4: keys 512..639)
                amix = sb.tile([128, NPAIR * PW], BF, name=f"amix_{it}", tag="amix", bufs=BPC)
                vmix = sb.tile([128, NPAIR * PW], BF, name=f"vmix_{it}", tag="vmix", bufs=BPC)
                nc.vector.tensor_copy(amix[:, :], vc_sb[4][:, :])
                nc.vector.tensor_copy(amix[0:1, :], v_sb[4][0:1, :])
                nc.vector.tensor_copy(vmix[:, :], v_sb[4][:, :])
                nc.vector.tensor_copy(vmix[0:1, :], vc_sb[4][0:1, :])
                if DEBUG_DUMP:
                    for c_ in range(NCH):
                        nc.sync.dma_start(dbg_v[it, 0, c_], v_sb[c_][:, :])
                        nc.sync.dma_start(dbg_v[it, 1, c_], vc_sb[c_][:, :])
                    nc.sync.dma_start(dbg_v[it, 2, 0], amix[:, :])
                    nc.sync.dma_start(dbg_v[it, 2, 1], vmix[:, :])

                # ==== per head pair: q/k projection, then attention ====
                oT = []
                for p in range(NPAIR):
                    dst = sb.tile([128, N], BF, name=f"qT_{it}_{p}",
                                  tag="qT", bufs=3)
                    for (qs, qw) in QP:
                        pp = ps.tile([128, 512], F32, name="pp", tag="ps_a", bufs=4)
                        for kc in range(KC):
                            nc.tensor.matmul(
                                pp[:, 0:qw],
                                lhsT=w_sb["wq"][kc][:, p * 128:(p + 1) * 128],
                                rhs=xT[(it, kc)][:, qs:qs + qw],
                                start=(kc == 0), stop=(kc == KC - 1),
                            )
                        pcopy(dst[:, qs:qs + qw], pp[:, 0:qw])
                    qTp = dst
                    # k^T per head, zero-padded to 128 partitions so S^T runs as
                    # a plain K=128 matmul
                    ke, ko = kper[p]
                    for (qs, qw) in QP:
                        pp = ps.tile([128, 512], F32, name="pp", tag="ps_a", bufs=4)
                        for kc in range(KC):
                            nc.tensor.matmul(
                                pp[:, 0:qw],
                                lhsT=w_sb["wk"][kc][:, p * 128:(p + 1) * 128],
                                rhs=xT[(it, kc)][:, qs:qs + qw],
                                start=(kc == 0), stop=(kc == KC - 1),
                            )
                        pcopy(ke[0:64, qs:qs + qw], pp[0:64, 0:qw])
                        pcopy(ko[64:128, qs:qs + qw], pp[64:128, 0:qw])
                    kTp = (ke, ko)

                    # S^T = scores transposed (keys on partitions), then exp.
                    # eA = q[0,512) (modality a), eB = q[512,906) (col 0 = q512,
                    # also modality a -- handled via the tail column of t2)
                    exps = {}
                    for c, (ks, ksz) in enumerate(KCH):
                        for par in range(2):
                            eA = sb.tile([128, 512], BF, name="eA", tag="expA", bufs=17)
                            eB = sb.tile([128, 394], BF, name="eB", tag="expB", bufs=17)
                            pst = ps.tile([128, 512], F32, name="pst", tag="ps_a", bufs=4)
                            nc.tensor.matmul(pst[0:ksz, 0:512],
                                             lhsT=kTp[par][:, ks:ks + ksz],
                                             rhs=qTp[:, 0:512], start=True, stop=True)
                            nc.scalar.activation(eA[0:ksz, :], pst[0:ksz, 0:512],
                                                 AF.Exp, scale=SCALE)
                            pst2 = ps.tile([128, 512], F32, name="pst2", tag="ps_a", bufs=4)
                            nc.tensor.matmul(pst2[0:ksz, 0:394],
                                             lhsT=kTp[par][:, ks:ks + ksz],
                                             rhs=qTp[:, 512:906], start=True, stop=True)
                            nc.scalar.activation(eB[0:ksz, 0:394], pst2[0:ksz, 0:394],
                                                 AF.Exp, scale=SCALE)
                            exps[(c, par)] = (eA, eB)

                    ot = sb.tile([128, N], BF, name=f"oT_{it}_{p}", tag="oT",
                                 bufs=NPAIR + 1)
                    for par in range(2):
                        t1 = ps.tile([128, 512], F32, name="t1", tag="ps_b", bufs=4)
                        t2 = ps.tile([128, 512], F32, name="t2", tag="ps_b", bufs=4)
                        if par == 0:
                            mrows = slice(0, 65)
                            csl = slice(p * PW, p * PW + 65)          # [V_even | 1]
                            drow, orows = 64, slice(0, 64)
                        else:
                            mrows = slice(0, 128)
                            csl = slice(p * PW + 66, p * PW + PW)     # [1 | 0*63 | V_odd]
                            drow, orows = 0, slice(64, 128)
                        # modality-a queries: q in [0,512) -> t1
                        for c, (ks, ksz) in enumerate(KCH):
                            va = amix if c == 4 else (v_sb[c] if c < 4 else vc_sb[c])
                            nc.tensor.matmul(t1[mrows, 0:512], lhsT=va[0:ksz, csl],
                                             rhs=exps[(c, par)][0][0:ksz, 0:512],
                                             start=(c == 0), stop=(c == NCH - 1),
                                             tile_position=(0, 0))
                        # modality-v queries: q in [512,906) -> t2 cols 0:394. Col 0
                        # (q=512) uses the wrong value set but its ones-column
                        # denominator row is value-independent, hence correct.
                        for c, (ks, ksz) in enumerate(KCH):
                            vv = vmix if c == 4 else (vc_sb[c] if c < 4 else v_sb[c])
                            nc.tensor.matmul(t2[mrows, 0:394], lhsT=vv[0:ksz, csl],
                                             rhs=exps[(c, par)][1][0:ksz, 0:394],
                                             start=(c == 0), stop=(c == NCH - 1),
                                             tile_position=(0, 0))
                        # q=512 is modality-a: recompute its value rows with the
                        # a-value set into the spare tail column of the same bank
                        for c, (ks, ksz) in enumerate(KCH):
                            va = amix if c == 4 else (v_sb[c] if c < 4 else vc_sb[c])
                            nc.tensor.matmul(t2[mrows, 394:395], lhsT=va[0:ksz, csl],
                                             rhs=exps[(c, par)][1][0:ksz, 0:1],
                                             start=(c == 0), stop=(c == NCH - 1),
                                             tile_position=(0, 0))
                        # softmax division: denominators sit in row `drow`. The
                        # custom DVE op only works from base partition 0, so for
                        # drow=64 run it over rows 0:65 (cost is column-bound; the
                        # extra rows are wasted lanes but free)
                        bcs = sb.tile([128, N], F32, name="bcs", tag="bc", bufs=2)
                        rsl = slice(0, drow + 1)
                        nc.vector.reciprocal_approx_fast(bcs[rsl, 0:512],
                                                         t1[rsl, 0:512])
                        nc.vector.reciprocal_approx_fast(bcs[rsl, 512:906],
                                                         t2[rsl, 0:394])
                        if drow != 0:
                            # hw partition_broadcast reads physical partition 0;
                            # relocate the reciprocal row there first
                            nc.sync.dma_start(bcs[0:1, 0:906], bcs[drow:drow + 1, 0:906])
                        bc2 = sb.tile([128, N], F32, name="bc2", tag="bc2", bufs=2)
                        nc.gpsimd.partition_broadcast(bc2[:, 0:906], bcs[0:1, 0:906])
                        nc.vector.tensor_mul(ot[orows, 0:512], t1[orows, 0:512], bc2[orows, 0:512])
                        nc.vector.tensor_mul(ot[orows, 513:906], t2[orows, 1:394], bc2[orows, 513:906])
                        nc.vector.tensor_mul(ot[orows, 512:513], t2[orows, 394:395], bc2[orows, 512:513])
                    oT.append(ot)

                # ============ output projection + bias ============
                for c, (ts, tsz) in enumerate(KCH):
                    for (cs, cw) in CPASS:
                        pp = ps.tile([128, 512], F32, name="pp", tag="ps_a", bufs=4)
                        for kp in range(NPAIR):
                            nc.tensor.matmul(
                                pp[0:tsz, 0:cw],
                                lhsT=oT[kp][:, ts:ts + tsz],
                                rhs=w_sb["wp"][kp][:, cs:cs + cw],
                                start=(kp == 0), stop=(kp == NPAIR - 1),
                            )
                        ob = sb.tile([128, 512], F32, name="ob", tag="ob", bufs=2)
                        nc.vector.tensor_add(ob[0:tsz, 0:cw], pp[0:tsz, 0:cw],
                                             bias_sb[0:tsz, cs:cs + cw])
                        # split the store into 32-row pieces so no single DMA
                        # occupies a queue long enough to head-of-line block the
                        # softmax-division row hops sharing the same queues
                        for rs in range(0, tsz, 32):
                            rw = min(32, tsz - rs)
                            nc.sync.dma_start(
                                out_d[it, ts + rs:ts + rs + rw, cs:cs + cw],
                                ob[rs:rs + rw, 0:cw])

    nc.compile()
    return nc


def _get_built():
    global _BUILT
    if _BUILT is None:
        _BUILT = _build()
    return _BUILT


def kernel(x, Wq, Wk, Wv, Wvc, Wp, bp):
    global LAST_RESULTS
    from concourse.bass_utils import run_bass_kernel_spmd

    x = np.asarray(x, dtype=np.float32)
    bf = ml_dtypes.bfloat16
    xT = np.ascontiguousarray(x.transpose(0, 2, 1)).astype(bf)      # (B, C, N)
    ws = {
        "wq": np.asarray(Wq, dtype=np.float32).astype(bf),
        "wk": np.asarray(Wk, dtype=np.float32).astype(bf),
        "wv": np.asarray(Wv, dtype=np.float32).astype(bf),
        "wvc": np.asarray(Wvc, dtype=np.float32).astype(bf),
        "wp": np.asarray(Wp, dtype=np.float32).astype(bf),
    }
    bias = np.ascontiguousarray(
        np.broadcast_to(np.asarray(bp, dtype=np.float32), (128, C))
    )

    if TRACE:
        _install_trace_shim()

    nc = _get_built()
    in_maps = []
    for i in range(N_CORES):
        m = {"xT": np.ascontiguousarray(xT[i * BPC:(i + 1) * BPC]), "bias": bias}
        m.update(ws)
        in_maps.append(m)

    res = run_bass_kernel_spmd(nc, in_maps, list(range(N_CORES)), trace=TRACE,
                               stitch_traces=False)
    LAST_RESULTS = res
    out = np.concatenate([res.results[i]["out"] for i in range(N_CORES)], axis=0)
    return out



# revision 57
# speedup vs baseline: 1.0417x; 1.0417x over previous
"""Multi-modality double-value attention on 8 TRN2 NeuronCores.

Sharding: data-parallel over batch (16 items -> 2 per core). Each core runs
the full attention block for its 2 items; weights are replicated. No
collectives. Host pre-transposes x to x^T and casts inputs to bf16; compute
is bf16 with fp32 PSUM accumulation; output is fp32.
"""

import numpy as np
import ml_dtypes

B, N, C = 16, 906, 768
H = 12
D = 64
M1 = 513
N_CORES = 8
BPC = B // N_CORES          # batch items per core
KC = C // 128               # 6 contraction chunks over C
NPAIR = H // 2              # 6 head pairs
NCH = (N + 127) // 128      # 8 key/token chunks over N
KCH = [(i * 128, min(128, N - i * 128)) for i in range(NCH)]
QP = [(0, 512), (512, N - 512)]      # column passes over N
CPASS = [(0, 512), (512, C - 512)]   # column passes over C
SCALE = D ** -0.5
PW = 194  # per-head-pair value block: [V_e(64) | 1 | 1 | 1 | 0*63 | V_o(64)]

TRACE = False          # set by test.py to capture a HW profile
DEBUG_DUMP = False     # add intermediate DRAM outputs (denominators, recips, oT)
LAST_RESULTS = None    # BassKernelResults of the most recent run

_BUILT = None


def _install_trace_shim():
    """The image's antenv lacks axon_hooks; recreate it so trace=True works."""
    import sys, types
    if "antenv.axon_hooks" in sys.modules:
        return
    mod = types.ModuleType("antenv.axon_hooks")
    mod._hook = None
    mod.set_axon_ntff_profile_hook = lambda h: setattr(mod, "_hook", h)
    mod.get_axon_ntff_profile_hook = lambda: mod._hook
    sys.modules["antenv.axon_hooks"] = mod
    import antenv
    antenv.axon_hooks = mod
    from trn_agent_boot.trn_boot import _ntff_profile_via_ctypes
    mod.set_axon_ntff_profile_hook(_ntff_profile_via_ctypes("/opt/axon/libaxon_pjrt.so"))


def _build():
    import concourse.tile as tile
    from concourse import bacc, bass_isa, mybir

    BF = mybir.dt.bfloat16
    F32 = mybir.dt.float32
    AF = mybir.ActivationFunctionType

    nc = bacc.Bacc("TRN2", target_bir_lowering=False, debug=False, num_devices=N_CORES)

    xT_d = nc.dram_tensor("xT", [BPC, C, N], BF, kind="ExternalInput").ap()
    w_d = {
        wn: nc.dram_tensor(wn, [C, C], BF, kind="ExternalInput").ap()
        for wn in ("wq", "wk", "wv", "wvc", "wp")
    }
    bias_d = nc.dram_tensor("bias", [128, C], F32, kind="ExternalInput").ap()
    out_d = nc.dram_tensor("out", [BPC, N, C], F32, kind="ExternalOutput").ap()
    if DEBUG_DUMP:
        dbg_rc = nc.dram_tensor("dbg_rc", [BPC, H, N], F32, kind="ExternalOutput").ap()
        dbg_ot = nc.dram_tensor("dbg_ot", [BPC, NPAIR, 128, N], BF, kind="ExternalOutput").ap()
        dbg_e = nc.dram_tensor("dbg_e", [BPC, 2, NCH, 128, N], BF, kind="ExternalOutput").ap()
        dbg_t1 = nc.dram_tensor("dbg_t1", [BPC, NPAIR, 2, 128, 512], F32, kind="ExternalOutput").ap()
        dbg_v = nc.dram_tensor("dbg_v", [BPC, 3, NCH, 128, NPAIR * PW], BF, kind="ExternalOutput").ap()

    with tile.TileContext(nc) as tc:
        from contextlib import ExitStack
        from concourse import library_config

        with ExitStack() as ctx:
            wpool = ctx.enter_context(tc.tile_pool(name="wpool", bufs=1))
            sb = ctx.enter_context(tc.tile_pool(name="sb", bufs=1))
            ps = ctx.enter_context(tc.tile_pool(name="ps", bufs=1, space="PSUM"))

            # partition_broadcast lives in the gpsimd 'attn' library; the
            # default 'standard' library executes it as garbage on HW
            nc.gpsimd.load_library(library_config.attn)

            # ---- constants: weights + bias ----
            # DMA issue order is tuned so the first projection's operands (wq +
            # item-0 x^T) land first; later weights/items stream in behind them
            w_sb = {}

            def load_w(wn):
                tiles = []
                for kc in range(KC):
                    t = wpool.tile([128, C], BF, name=f"{wn}_{kc}", tag=f"{wn}_{kc}")
                    nc.sync.dma_start(t[:], w_d[wn][kc * 128:(kc + 1) * 128, :])
                    tiles.append(t)
                w_sb[wn] = tiles

            load_w("wq")
            bias_sb = wpool.tile([128, C], F32, name="bias_sb", tag="bias_sb")

            # persistent zero-padded k^T tiles, shared across batch items; the
            # pad halves are zeroed once and never rewritten. Scores run as
            # plain K=128 matmuls (row tiling is ~2x slower per MM on this HW)
            kper = []
            for t_ in range(NPAIR):
                ke = wpool.tile([128, N], BF, name=f"kTh_e{t_}", tag=f"kTh_e{t_}")
                ko = wpool.tile([128, N], BF, name=f"kTh_o{t_}", tag=f"kTh_o{t_}")
                nc.vector.memset(ke[64:128, :], 0.0)
                nc.vector.memset(ko[0:64, :], 0.0)
                kper.append((ke, ko))

            # ---- x^T tiles, both items prefetched ----
            xT = {}
            for kc in range(KC):
                t = sb.tile([128, N], BF, name=f"xT_0_{kc}", tag="xT", bufs=6)
                nc.sync.dma_start(t[:], xT_d[0, kc * 128:(kc + 1) * 128, :])
                xT[(0, kc)] = t
            load_w("wk")
            load_w("wv")
            load_w("wvc")
            for kc in range(KC):
                t = sb.tile([128, N], BF, name=f"xT_1_{kc}", tag="xT", bufs=6)
                nc.sync.dma_start(t[:], xT_d[1, kc * 128:(kc + 1) * 128, :])
                xT[(1, kc)] = t
            load_w("wp")
            nc.sync.dma_start(bias_sb[:], bias_d[:])

            for it in range(BPC):
                pcopy = nc.vector.tensor_copy
                # ============ projections ============
                qT, kTh = [], []
                for t_ in range(NPAIR):
                    dst = sb.tile([128, N], BF, name=f"qT_{it}_{t_}",
                                  tag="qT", bufs=NPAIR + 1)
                    for (qs, qw) in QP:
                        pp = ps.tile([128, 512], F32, name="pp", tag="ps_a", bufs=4)
                        for kc in range(KC):
                            nc.tensor.matmul(
                                pp[:, 0:qw],
                                lhsT=w_sb["wq"][kc][:, t_ * 128:(t_ + 1) * 128],
                                rhs=xT[(it, kc)][:, qs:qs + qw],
                                start=(kc == 0), stop=(kc == KC - 1),
                            )
                        pcopy(dst[:, qs:qs + qw], pp[:, 0:qw])
                    qT.append(dst)
                    # k^T per head, zero-padded to 128 partitions so S^T runs as
                    # a plain K=128 matmul
                    ke, ko = kper[t_]
                    for (qs, qw) in QP:
                        pp = ps.tile([128, 512], F32, name="pp", tag="ps_a", bufs=4)
                        for kc in range(KC):
                            nc.tensor.matmul(
                                pp[:, 0:qw],
                                lhsT=w_sb["wk"][kc][:, t_ * 128:(t_ + 1) * 128],
                                rhs=xT[(it, kc)][:, qs:qs + qw],
                                start=(kc == 0), stop=(kc == KC - 1),
                            )
                        pcopy(ke[0:64, qs:qs + qw], pp[0:64, 0:qw])
                        pcopy(ko[64:128, qs:qs + qw], pp[64:128, 0:qw])
                    kTh.append(ke)
                    kTh.append(ko)

                v_sb, vc_sb = [], []
                for c, (ts, tsz) in enumerate(KCH):
                    for dst_list, wn, tg in ((v_sb, "wv", "v"), (vc_sb, "wvc", "vc")):
                        dst = sb.tile([128, NPAIR * PW], BF, name=f"{tg}_{it}_{c}",
                                      tag=tg, bufs=NCH + 1)
                        if tsz < 128:
                            # stationary loads may read all 128 partitions; keep
                            # the unwritten tail finite
                            nc.vector.memset(dst[:, :], 0.0)
                        dvw = dst[0:tsz, :].rearrange("p (g c) -> p g c", c=PW)
                        for (cs, cw) in CPASS:
                            pp = ps.tile([128, 512], F32, name="pp", tag="ps_a", bufs=4)
                            for kc in range(KC):
                                nc.tensor.matmul(
                                    pp[0:tsz, 0:cw],
                                    lhsT=xT[(it, kc)][:, ts:ts + tsz],
                                    rhs=w_sb[wn][kc][:, cs:cs + cw],
                                    start=(kc == 0), stop=(kc == KC - 1),
                                )
                            g0, gn = (0, 4) if cs == 0 else (4, 2)
                            src = pp[0:tsz, 0:cw].rearrange("p (g r d) -> p g r d", r=2, d=D)
                            pcopy(dvw[:, g0:g0 + gn, 0:D], src[:, :, 0, :])
                            pcopy(dvw[:, g0:g0 + gn, 130:194], src[:, :, 1, :])
                        # cols 64 and 66 are the denominator ones-columns; cols
                        # 67:130 feed PSUM rows that are never read, so they can
                        # stay stale
                        nc.vector.memset(dvw[:, :, 64:67], 1.0)
                        dst_list.append(dst)

                # mixed tiles for the key chunk straddling M1 (chunk 4: keys 512..639)
                amix = sb.tile([128, NPAIR * PW], BF, name=f"amix_{it}", tag="amix", bufs=BPC)
                vmix = sb.tile([128, NPAIR * PW], BF, name=f"vmix_{it}", tag="vmix", bufs=BPC)
                nc.vector.tensor_copy(amix[:, :], vc_sb[4][:, :])
                nc.vector.tensor_copy(amix[0:1, :], v_sb[4][0:1, :])
                nc.vector.tensor_copy(vmix[:, :], v_sb[4][:, :])
                nc.vector.tensor_copy(vmix[0:1, :], vc_sb[4][0:1, :])
                if DEBUG_DUMP:
                    for c_ in range(NCH):
                        nc.sync.dma_start(dbg_v[it, 0, c_], v_sb[c_][:, :])
                        nc.sync.dma_start(dbg_v[it, 1, c_], vc_sb[c_][:, :])
                    nc.sync.dma_start(dbg_v[it, 2, 0], amix[:, :])
                    nc.sync.dma_start(dbg_v[it, 2, 1], vmix[:, :])

                # ============ attention, one head pair at a time ============
                oT = []
                for p in range(NPAIR):
                    # S^T = scores transposed (keys on partitions), then exp.
                    # eA = q[0,512) (modality a), eB = q[512,906) (col 0 = q512,
                    # also modality a -- handled via the tail column of t2)
                    exps = {}
                    for c, (ks, ksz) in enumerate(KCH):
                        for par in range(2):
                            eA = sb.tile([128, 512], BF, name="eA", tag="expA", bufs=17)
                            eB = sb.tile([128, 394], BF, name="eB", tag="expB", bufs=17)
                            pst = ps.tile([128, 512], F32, name="pst", tag="ps_a", bufs=4)
                            nc.tensor.matmul(pst[0:ksz, 0:512],
                                             lhsT=kTh[2 * p + par][:, ks:ks + ksz],
                                             rhs=qT[p][:, 0:512], start=True, stop=True)
                            nc.scalar.activation(eA[0:ksz, :], pst[0:ksz, 0:512],
                                                 AF.Exp, scale=SCALE)
                            pst2 = ps.tile([128, 512], F32, name="pst2", tag="ps_a", bufs=4)
                            nc.tensor.matmul(pst2[0:ksz, 0:394],
                                             lhsT=kTh[2 * p + par][:, ks:ks + ksz],
                                             rhs=qT[p][:, 512:906], start=True, stop=True)
                            nc.scalar.activation(eB[0:ksz, 0:394], pst2[0:ksz, 0:394],
                                                 AF.Exp, scale=SCALE)
                            exps[(c, par)] = (eA, eB)

                    ot = sb.tile([128, N], BF, name=f"oT_{it}_{p}", tag="oT",
                                 bufs=NPAIR + 1)
                    for par in range(2):
                        t1 = ps.tile([128, 512], F32, name="t1", tag="ps_b", bufs=4)
                        t2 = ps.tile([128, 512], F32, name="t2", tag="ps_b", bufs=4)
                        if par == 0:
                            mrows = slice(0, 65)
                            csl = slice(p * PW, p * PW + 65)          # [V_even | 1]
                            drow, orows = 64, slice(0, 64)
                        else:
                            mrows = slice(0, 128)
                            csl = slice(p * PW + 66, p * PW + PW)     # [1 | 0*63 | V_odd]
                            drow, orows = 0, slice(64, 128)
                        # modality-a queries: q in [0,512) -> t1
                        for c, (ks, ksz) in enumerate(KCH):
                            va = amix if c == 4 else (v_sb[c] if c < 4 else vc_sb[c])
                            nc.tensor.matmul(t1[mrows, 0:512], lhsT=va[0:ksz, csl],
                                             rhs=exps[(c, par)][0][0:ksz, 0:512],
                                             start=(c == 0), stop=(c == NCH - 1),
                                             tile_position=(0, 0))
                        # modality-v queries: q in [512,906) -> t2 cols 0:394. Col 0
                        # (q=512) uses the wrong value set but its ones-column
                        # denominator row is value-independent, hence correct.
                        for c, (ks, ksz) in enumerate(KCH):
                            vv = vmix if c == 4 else (vc_sb[c] if c < 4 else v_sb[c])
                            nc.tensor.matmul(t2[mrows, 0:394], lhsT=vv[0:ksz, csl],
                                             rhs=exps[(c, par)][1][0:ksz, 0:394],
                                             start=(c == 0), stop=(c == NCH - 1),
                                             tile_position=(0, 0))
                        # q=512 is modality-a: recompute its value rows with the
                        # a-value set into the spare tail column of the same bank
                        for c, (ks, ksz) in enumerate(KCH):
                            va = amix if c == 4 else (v_sb[c] if c < 4 else vc_sb[c])
                            nc.tensor.matmul(t2[mrows, 394:395], lhsT=va[0:ksz, csl],
                                             rhs=exps[(c, par)][1][0:ksz, 0:1],
                                             start=(c == 0), stop=(c == NCH - 1),
                                             tile_position=(0, 0))
                        # softmax division: denominators sit in row `drow`. The
                        # custom DVE op only works from base partition 0, so for
                        # drow=64 run it over rows 0:65 (cost is column-bound; the
                        # extra rows are wasted lanes but free)
                        bcs = sb.tile([128, N], F32, name="bcs", tag="bc", bufs=2)
                        rsl = slice(0, drow + 1)
                        nc.vector.reciprocal_approx_fast(bcs[rsl, 0:512],
                                                         t1[rsl, 0:512])
                        nc.vector.reciprocal_approx_fast(bcs[rsl, 512:906],
                                                         t2[rsl, 0:394])
                        if drow != 0:
                            # hw partition_broadcast reads physical partition 0;
                            # relocate the reciprocal row there first
                            nc.sync.dma_start(bcs[0:1, 0:906], bcs[drow:drow + 1, 0:906])
                        bc2 = sb.tile([128, N], F32, name="bc2", tag="bc2", bufs=2)
                        nc.gpsimd.partition_broadcast(bc2[:, 0:906], bcs[0:1, 0:906])
                        nc.vector.tensor_mul(ot[orows, 0:512], t1[orows, 0:512], bc2[orows, 0:512])
                        nc.vector.tensor_mul(ot[orows, 513:906], t2[orows, 1:394], bc2[orows, 513:906])
                        nc.vector.tensor_mul(ot[orows, 512:513], t2[orows, 394:395], bc2[orows, 512:513])
                    oT.append(ot)

                # ============ output projection + bias ============
                for c, (ts, tsz) in enumerate(KCH):
                    for (cs, cw) in CPASS:
                        pp = ps.tile([128, 512], F32, name="pp", tag="ps_a", bufs=4)
                        for kp in range(NPAIR):
                            nc.tensor.matmul(
                                pp[0:tsz, 0:cw],
                                lhsT=oT[kp][:, ts:ts + tsz],
                                rhs=w_sb["wp"][kp][:, cs:cs + cw],
                                start=(kp == 0), stop=(kp == NPAIR - 1),
                            )
                        ob = sb.tile([128, 512], F32, name="ob", tag="ob", bufs=2)
                        nc.vector.tensor_add(ob[0:tsz, 0:cw], pp[0:tsz, 0:cw],
                                             bias_sb[0:tsz, cs:cs + cw])
                        # split the store into 32-row pieces so no single DMA
                        # occupies a queue long enough to head-of-line block the
                        # softmax-division row hops sharing the same queues
                        for rs in range(0, tsz, 32):
                            rw = min(32, tsz - rs)
                            nc.sync.dma_start(
                                out_d[it, ts + rs:ts + rs + rw, cs:cs + cw],
                                ob[rs:rs + rw, 0:cw])

    nc.compile()
    return nc


def _get_built():
    global _BUILT
    if _BUILT is None:
        _BUILT = _build()
    return _BUILT


def kernel(x, Wq, Wk, Wv, Wvc, Wp, bp):
    global LAST_RESULTS
    from concourse.bass_utils import run_bass_kernel_spmd

    x = np.asarray(x, dtype=np.float32)
    bf = ml_dtypes.bfloat16
    xT = np.ascontiguousarray(x.transpose(0, 2, 1)).astype(bf)      # (B, C, N)
    ws = {
        "wq": np.asarray(Wq, dtype=np.float32).astype(bf),
        "wk": np.asarray(Wk, dtype=np.float32).astype(bf),
        "wv": np.asarray(Wv, dtype=np.float32).astype(bf),
        "wvc": np.asarray(Wvc, dtype=np.float32).astype(bf),
        "wp": np.asarray(Wp, dtype=np.float32).astype(bf),
    }
    bias = np.ascontiguousarray(
        np.broadcast_to(np.asarray(bp, dtype=np.float32), (128, C))
    )

    if TRACE:
        _install_trace_shim()

    nc = _get_built()
    in_maps = []
    for i in range(N_CORES):
        m = {"xT": np.ascontiguousarray(xT[i * BPC:(i + 1) * BPC]), "bias": bias}
        m.update(ws)
        in_maps.append(m)

    res = run_bass_kernel_spmd(nc, in_maps, list(range(N_CORES)), trace=TRACE,
                               stitch_traces=False)
    LAST_RESULTS = res
    out = np.concatenate([res.results[i]["out"] for i in range(N_CORES)], axis=0)
    return out



# revision 59
# speedup vs baseline: 1.1010x; 1.0570x over previous
"""Multi-modality double-value attention on 8 TRN2 NeuronCores.

Sharding: data-parallel over batch (16 items -> 2 per core). Each core runs
the full attention block for its 2 items; weights are replicated. No
collectives. Host pre-transposes x to x^T and casts inputs to bf16; compute
is bf16 with fp32 PSUM accumulation; output is fp32.
"""

import numpy as np
import ml_dtypes

B, N, C = 16, 906, 768
H = 12
D = 64
M1 = 513
N_CORES = 8
BPC = B // N_CORES          # batch items per core
KC = C // 128               # 6 contraction chunks over C
NPAIR = H // 2              # 6 head pairs
NCH = (N + 127) // 128      # 8 key/token chunks over N
KCH = [(i * 128, min(128, N - i * 128)) for i in range(NCH)]
QP = [(0, 512), (512, N - 512)]      # column passes over N
CPASS = [(0, 512), (512, C - 512)]   # column passes over C
SCALE = D ** -0.5
PW = 194  # per-head-pair value block: [V_e(64) | 1 | 1 | 1 | 0*63 | V_o(64)]

TRACE = False          # set by test.py to capture a HW profile
DEBUG_DUMP = False     # add intermediate DRAM outputs (denominators, recips, oT)
LAST_RESULTS = None    # BassKernelResults of the most recent run

_BUILT = None


def _install_trace_shim():
    """The image's antenv lacks axon_hooks; recreate it so trace=True works."""
    import sys, types
    if "antenv.axon_hooks" in sys.modules:
        return
    mod = types.ModuleType("antenv.axon_hooks")
    mod._hook = None
    mod.set_axon_ntff_profile_hook = lambda h: setattr(mod, "_hook", h)
    mod.get_axon_ntff_profile_hook = lambda: mod._hook
    sys.modules["antenv.axon_hooks"] = mod
    import antenv
    antenv.axon_hooks = mod
    from trn_agent_boot.trn_boot import _ntff_profile_via_ctypes
    mod.set_axon_ntff_profile_hook(_ntff_profile_via_ctypes("/opt/axon/libaxon_pjrt.so"))


def _build():
    import concourse.tile as tile
    from concourse import bacc, bass_isa, mybir

    BF = mybir.dt.bfloat16
    F32 = mybir.dt.float32
    AF = mybir.ActivationFunctionType

    nc = bacc.Bacc("TRN2", target_bir_lowering=False, debug=False, num_devices=N_CORES)

    xT_d = nc.dram_tensor("xT", [BPC, C, N], BF, kind="ExternalInput").ap()
    w_d = {
        wn: nc.dram_tensor(wn, [C, C], BF, kind="ExternalInput").ap()
        for wn in ("wq", "wk", "wv", "wvc", "wp")
    }
    bias_d = nc.dram_tensor("bias", [128, C], F32, kind="ExternalInput").ap()
    out_d = nc.dram_tensor("out", [BPC, N, C], F32, kind="ExternalOutput").ap()
    if DEBUG_DUMP:
        dbg_rc = nc.dram_tensor("dbg_rc", [BPC, H, N], F32, kind="ExternalOutput").ap()
        dbg_ot = nc.dram_tensor("dbg_ot", [BPC, NPAIR, 128, N], BF, kind="ExternalOutput").ap()
        dbg_e = nc.dram_tensor("dbg_e", [BPC, 2, NCH, 128, N], BF, kind="ExternalOutput").ap()
        dbg_t1 = nc.dram_tensor("dbg_t1", [BPC, NPAIR, 2, 128, 512], F32, kind="ExternalOutput").ap()
        dbg_v = nc.dram_tensor("dbg_v", [BPC, 3, NCH, 128, NPAIR * PW], BF, kind="ExternalOutput").ap()

    with tile.TileContext(nc) as tc:
        from contextlib import ExitStack
        from concourse import library_config

        with ExitStack() as ctx:
            wpool = ctx.enter_context(tc.tile_pool(name="wpool", bufs=1))
            sb = ctx.enter_context(tc.tile_pool(name="sb", bufs=1))
            ps = ctx.enter_context(tc.tile_pool(name="ps", bufs=1, space="PSUM"))

            # partition_broadcast lives in the gpsimd 'attn' library; the
            # default 'standard' library executes it as garbage on HW
            nc.gpsimd.load_library(library_config.attn)

            # ---- constants: weights + bias ----
            # DMA issue order is tuned so the first projection's operands (wq +
            # item-0 x^T) land first; later weights/items stream in behind them
            w_sb = {}

            def load_w(wn):
                tiles = []
                for kc in range(KC):
                    t = wpool.tile([128, C], BF, name=f"{wn}_{kc}", tag=f"{wn}_{kc}")
                    nc.sync.dma_start(t[:], w_d[wn][kc * 128:(kc + 1) * 128, :])
                    tiles.append(t)
                w_sb[wn] = tiles

            load_w("wq")
            bias_sb = wpool.tile([128, C], F32, name="bias_sb", tag="bias_sb")

            # persistent zero-padded k^T tiles, shared across batch items; the
            # pad halves are zeroed once and never rewritten. Scores run as
            # plain K=128 matmuls (row tiling is ~2x slower per MM on this HW)
            kper = []
            for t_ in range(NPAIR):
                ke = wpool.tile([128, N], BF, name=f"kTh_e{t_}", tag=f"kTh_e{t_}")
                ko = wpool.tile([128, N], BF, name=f"kTh_o{t_}", tag=f"kTh_o{t_}")
                nc.vector.memset(ke[64:128, :], 0.0)
                nc.vector.memset(ko[0:64, :], 0.0)
                kper.append((ke, ko))

            # ---- x^T tiles, both items prefetched ----
            xT = {}
            for kc in range(KC):
                t = sb.tile([128, N], BF, name=f"xT_0_{kc}", tag="xT", bufs=6)
                nc.sync.dma_start(t[:], xT_d[0, kc * 128:(kc + 1) * 128, :])
                xT[(0, kc)] = t
            load_w("wk")
            load_w("wv")
            load_w("wvc")
            for kc in range(KC):
                t = sb.tile([128, N], BF, name=f"xT_1_{kc}", tag="xT", bufs=6)
                nc.sync.dma_start(t[:], xT_d[1, kc * 128:(kc + 1) * 128, :])
                xT[(1, kc)] = t
            load_w("wp")
            nc.sync.dma_start(bias_sb[:], bias_d[:])

            for it in range(BPC):
                pcopy = nc.vector.tensor_copy
                # ============ projections ============
                qT, kTh = [], []
                for t_ in range(NPAIR):
                    dst = sb.tile([128, N], BF, name=f"qT_{it}_{t_}",
                                  tag="qT", bufs=NPAIR + 1)
                    for (qs, qw) in QP:
                        pp = ps.tile([128, 512], F32, name="pp", tag="ps_a", bufs=3)
                        for kc in range(KC):
                            nc.tensor.matmul(
                                pp[:, 0:qw],
                                lhsT=w_sb["wq"][kc][:, t_ * 128:(t_ + 1) * 128],
                                rhs=xT[(it, kc)][:, qs:qs + qw],
                                start=(kc == 0), stop=(kc == KC - 1),
                            )
                        pcopy(dst[:, qs:qs + qw], pp[:, 0:qw])
                    qT.append(dst)
                    # k^T per head, zero-padded to 128 partitions so S^T runs as
                    # a plain K=128 matmul
                    ke, ko = kper[t_]
                    for (qs, qw) in QP:
                        pp = ps.tile([128, 512], F32, name="pp", tag="ps_a", bufs=3)
                        for kc in range(KC):
                            nc.tensor.matmul(
                                pp[:, 0:qw],
                                lhsT=w_sb["wk"][kc][:, t_ * 128:(t_ + 1) * 128],
                                rhs=xT[(it, kc)][:, qs:qs + qw],
                                start=(kc == 0), stop=(kc == KC - 1),
                            )
                        pcopy(ke[0:64, qs:qs + qw], pp[0:64, 0:qw])
                        pcopy(ko[64:128, qs:qs + qw], pp[64:128, 0:qw])
                    kTh.append(ke)
                    kTh.append(ko)

                v_sb, vc_sb = [], []
                for c, (ts, tsz) in enumerate(KCH):
                    for dst_list, wn, tg in ((v_sb, "wv", "v"), (vc_sb, "wvc", "vc")):
                        dst = sb.tile([128, NPAIR * PW], BF, name=f"{tg}_{it}_{c}",
                                      tag=tg, bufs=NCH + 1)
                        if tsz < 128:
                            # stationary loads may read all 128 partitions; keep
                            # the unwritten tail finite
                            nc.vector.memset(dst[:, :], 0.0)
                        dvw = dst[0:tsz, :].rearrange("p (g c) -> p g c", c=PW)
                        for (cs, cw) in CPASS:
                            pp = ps.tile([128, 512], F32, name="pp", tag="ps_a", bufs=3)
                            for kc in range(KC):
                                nc.tensor.matmul(
                                    pp[0:tsz, 0:cw],
                                    lhsT=xT[(it, kc)][:, ts:ts + tsz],
                                    rhs=w_sb[wn][kc][:, cs:cs + cw],
                                    start=(kc == 0), stop=(kc == KC - 1),
                                )
                            g0, gn = (0, 4) if cs == 0 else (4, 2)
                            src = pp[0:tsz, 0:cw].rearrange("p (g r d) -> p g r d", r=2, d=D)
                            pcopy(dvw[:, g0:g0 + gn, 0:D], src[:, :, 0, :])
                            pcopy(dvw[:, g0:g0 + gn, 130:194], src[:, :, 1, :])
                        # cols 64 and 66 are the denominator ones-columns; cols
                        # 67:130 feed PSUM rows that are never read, so they can
                        # stay stale
                        nc.vector.memset(dvw[:, :, 64:67], 1.0)
                        dst_list.append(dst)

                # mixed tiles for the key chunk straddling M1 (chunk 4: keys 512..639)
                amix = sb.tile([128, NPAIR * PW], BF, name=f"amix_{it}", tag="amix", bufs=BPC)
                vmix = sb.tile([128, NPAIR * PW], BF, name=f"vmix_{it}", tag="vmix", bufs=BPC)
                nc.vector.tensor_copy(amix[:, :], vc_sb[4][:, :])
                nc.vector.tensor_copy(amix[0:1, :], v_sb[4][0:1, :])
                nc.vector.tensor_copy(vmix[:, :], v_sb[4][:, :])
                nc.vector.tensor_copy(vmix[0:1, :], vc_sb[4][0:1, :])
                if DEBUG_DUMP:
                    for c_ in range(NCH):
                        nc.sync.dma_start(dbg_v[it, 0, c_], v_sb[c_][:, :])
                        nc.sync.dma_start(dbg_v[it, 1, c_], vc_sb[c_][:, :])
                    nc.sync.dma_start(dbg_v[it, 2, 0], amix[:, :])
                    nc.sync.dma_start(dbg_v[it, 2, 1], vmix[:, :])

                # ============ attention, one head pair at a time ============
                oT = []
                for p in range(NPAIR):
                    # S^T = scores transposed (keys on partitions), then exp.
                    # eA = q[0,512) (modality a), eB = q[512,906) (col 0 = q512,
                    # also modality a -- handled via the tail column of t2)
                    exps = {}
                    for c, (ks, ksz) in enumerate(KCH):
                        for par in range(2):
                            eA = sb.tile([128, 512], BF, name="eA", tag="expA", bufs=17)
                            eB = sb.tile([128, 394], BF, name="eB", tag="expB", bufs=17)
                            pst = ps.tile([128, 512], F32, name="pst", tag="ps_a", bufs=3)
                            nc.tensor.matmul(pst[0:ksz, 0:512],
                                             lhsT=kTh[2 * p + par][:, ks:ks + ksz],
                                             rhs=qT[p][:, 0:512], start=True, stop=True)
                            nc.scalar.activation(eA[0:ksz, :], pst[0:ksz, 0:512],
                                                 AF.Exp, scale=SCALE)
                            pst2 = ps.tile([128, 512], F32, name="pst2", tag="ps_a", bufs=3)
                            nc.tensor.matmul(pst2[0:ksz, 0:394],
                                             lhsT=kTh[2 * p + par][:, ks:ks + ksz],
                                             rhs=qT[p][:, 512:906], start=True, stop=True)
                            nc.scalar.activation(eB[0:ksz, 0:394], pst2[0:ksz, 0:394],
                                                 AF.Exp, scale=SCALE)
                            exps[(c, par)] = (eA, eB)

                    ot = sb.tile([128, N], BF, name=f"oT_{it}_{p}", tag="oT",
                                 bufs=NPAIR + 1)
                    for par in range(2):
                        t1 = ps.tile([128, 512], F32, name="t1", tag="ps_b", bufs=5)
                        t2 = ps.tile([128, 512], F32, name="t2", tag="ps_b", bufs=5)
                        if par == 0:
                            mrows = slice(0, 65)
                            csl = slice(p * PW, p * PW + 65)          # [V_even | 1]
                            drow, orows = 64, slice(0, 64)
                        else:
                            mrows = slice(0, 128)
                            csl = slice(p * PW + 66, p * PW + PW)     # [1 | 0*63 | V_odd]
                            drow, orows = 0, slice(64, 128)
                        # modality-a queries: q in [0,512) -> t1
                        for c, (ks, ksz) in enumerate(KCH):
                            va = amix if c == 4 else (v_sb[c] if c < 4 else vc_sb[c])
                            nc.tensor.matmul(t1[mrows, 0:512], lhsT=va[0:ksz, csl],
                                             rhs=exps[(c, par)][0][0:ksz, 0:512],
                                             start=(c == 0), stop=(c == NCH - 1),
                                             tile_position=(0, 0))
                        # modality-v queries: q in [512,906) -> t2 cols 0:394. Col 0
                        # (q=512) uses the wrong value set but its ones-column
                        # denominator row is value-independent, hence correct.
                        for c, (ks, ksz) in enumerate(KCH):
                            vv = vmix if c == 4 else (vc_sb[c] if c < 4 else v_sb[c])
                            nc.tensor.matmul(t2[mrows, 0:394], lhsT=vv[0:ksz, csl],
                                             rhs=exps[(c, par)][1][0:ksz, 0:394],
                                             start=(c == 0), stop=(c == NCH - 1),
                                             tile_position=(0, 0))
                        # q=512 is modality-a: recompute its value rows with the
                        # a-value set into the spare tail column of the same bank
                        for c, (ks, ksz) in enumerate(KCH):
                            va = amix if c == 4 else (v_sb[c] if c < 4 else vc_sb[c])
                            nc.tensor.matmul(t2[mrows, 394:395], lhsT=va[0:ksz, csl],
                                             rhs=exps[(c, par)][1][0:ksz, 0:1],
                                             start=(c == 0), stop=(c == NCH - 1),
                                             tile_position=(0, 0))
                        # softmax division: denominators sit in row `drow`. The
                        # custom DVE op only works from base partition 0, so for
                        # drow=64 run it over rows 0:65 (cost is column-bound; the
                        # extra rows are wasted lanes but free)
                        bcs = sb.tile([128, N], F32, name="bcs", tag="bc", bufs=2)
                        rsl = slice(0, drow + 1)
                        nc.vector.reciprocal_approx_fast(bcs[rsl, 0:512],
                                                         t1[rsl, 0:512])
                        nc.vector.reciprocal_approx_fast(bcs[rsl, 512:906],
                                                         t2[rsl, 0:394])
                        if drow != 0:
                            # hw partition_broadcast reads physical partition 0;
                            # relocate the reciprocal row there first
                            nc.sync.dma_start(bcs[0:1, 0:906], bcs[drow:drow + 1, 0:906])
                        bc2 = sb.tile([128, N], F32, name="bc2", tag="bc2", bufs=2)
                        nc.gpsimd.partition_broadcast(bc2[:, 0:906], bcs[0:1, 0:906])
                        nc.vector.tensor_mul(ot[orows, 0:512], t1[orows, 0:512], bc2[orows, 0:512])
                        nc.vector.tensor_mul(ot[orows, 513:906], t2[orows, 1:394], bc2[orows, 513:906])
                        nc.vector.tensor_mul(ot[orows, 512:513], t2[orows, 394:395], bc2[orows, 512:513])
                    oT.append(ot)

                # ============ output projection + bias ============
                for c, (ts, tsz) in enumerate(KCH):
                    for (cs, cw) in CPASS:
                        pp = ps.tile([128, 512], F32, name="pp", tag="ps_a", bufs=3)
                        for kp in range(NPAIR):
                            nc.tensor.matmul(
                                pp[0:tsz, 0:cw],
                                lhsT=oT[kp][:, ts:ts + tsz],
                                rhs=w_sb["wp"][kp][:, cs:cs + cw],
                                start=(kp == 0), stop=(kp == NPAIR - 1),
                            )
                        ob = sb.tile([128, 512], F32, name="ob", tag="ob", bufs=2)
                        nc.vector.tensor_add(ob[0:tsz, 0:cw], pp[0:tsz, 0:cw],
                                             bias_sb[0:tsz, cs:cs + cw])
                        nc.sync.dma_start(out_d[it, ts:ts + tsz, cs:cs + cw], ob[0:tsz, 0:cw])

    nc.compile()
    return nc


def _get_built():
    global _BUILT
    if _BUILT is None:
        _BUILT = _build()
    return _BUILT


def kernel(x, Wq, Wk, Wv, Wvc, Wp, bp):
    global LAST_RESULTS
    from concourse.bass_utils import run_bass_kernel_spmd

    x = np.asarray(x, dtype=np.float32)
    bf = ml_dtypes.bfloat16
    xT = np.ascontiguousarray(x.transpose(0, 2, 1)).astype(bf)      # (B, C, N)
    ws = {
        "wq": np.asarray(Wq, dtype=np.float32).astype(bf),
        "wk": np.asarray(Wk, dtype=np.float32).astype(bf),
        "wv": np.asarray(Wv, dtype=np.float32).astype(bf),
        "wvc": np.asarray(Wvc, dtype=np.float32).astype(bf),
        "wp": np.asarray(Wp, dtype=np.float32).astype(bf),
    }
    bias = np.ascontiguousarray(
        np.broadcast_to(np.asarray(bp, dtype=np.float32), (128, C))
    )

    if TRACE:
        _install_trace_shim()

    nc = _get_built()
    in_maps = []
    for i in range(N_CORES):
        m = {"xT": np.ascontiguousarray(xT[i * BPC:(i + 1) * BPC]), "bias": bias}
        m.update(ws)
        in_maps.append(m)

    res = run_bass_kernel_spmd(nc, in_maps, list(range(N_CORES)), trace=TRACE,
                               stitch_traces=False)
    LAST_RESULTS = res
    out = np.concatenate([res.results[i]["out"] for i in range(N_CORES)], axis=0)
    return out

